# revision 36
# baseline (speedup 1.0000x reference)
"""Trainium2 Bass kernel for DiagonalS5SSM.

Math (per batch b, with the reference's where(valid,...) elided — valid is a
prefix mask in l and the output is masked by the same prefix, so the frozen
tail states never reach the output):

    it[l, n]  = sum_d x[b, l, d] * bbar[n, d]          (complex)
    s[l, n]   = abar[n] * s[l-1, n] + it[l, n]         (complex scan)
    y[b, l, :] = mask[l] * (Re(s[l] @ c^T) + x[b, l] @ D^T)

The complex scan is decoupled into two real scans via polar form
abar = rho * e^{i theta} (rho < 1, so no dynamic-range blowup):

    v[l] = e^{-i theta l} * it[l]       (elementwise rotation)
    w[l] = rho * w[l-1] + v[l]          (HW tensor_tensor_scan per plane)
    s[l] = e^{+i theta l} * w[l]        (rotation back)

Sharding: data-parallel over batch, 2 batches per core, no collectives.
Matmuls and elementwise rotations run in bf16 (separate LDWEIGHTS pipelines
under the matmul via the PE reorder window; DVE gets its 2x packed mode);
the scan coefficient rho stays fp32 (decay errors would compound over 2048
steps) and the scan state is fp32 internally in hardware. x is
pre-transposed on the host so the contraction dim lands on partitions
without PE transposes. The whole pipeline is chunk-streamed in 512-row
chunks with the two per-batch scan chains interleaved; the scan chains
across chunks via initial=prev[:, -1:]. The final y = mask*resp + x@D^T
uses an ACT masked copy plus a gpsimd accumulate-DMA of the host-prepared
x@D^T term (identity D short-circuits the host matmul).
"""

import time

import numpy as np

import concourse.bass as bass
import concourse.tile as tile
from concourse import bacc, mybir
from concourse.bass_utils import run_bass_kernel_spmd

B, L, D, N = 16, 2048, 512, 256
NCORES = 8
BL = B // NCORES          # batches per core
R = BL * L                # rows per core (r = b*L + l)
NH = N // 128             # n-halves
DK = D // 128             # d-chunks
RT = R // 128             # 128-row tiles per core
RCH = R // 512            # 512-row chunks per core
CPB = L // 512            # chunks per batch

F32 = mybir.dt.float32
BF16 = mybir.dt.bfloat16
MM_DT = mybir.dt.float32r  # matmul operand dtype (float32r: 1 cyc/row)
EW_DT = BF16               # elementwise/rotation dtype (2x DVE mode)
S_DT = BF16                # s-plane / stage-5 matmul dtype

AluOp = mybir.AluOpType
ACT_COPY = mybir.ActivationFunctionType.Copy


def _bcast_cols(ap: bass.AP, n: int) -> bass.AP:
    """[128, 1] AP -> [128, n] free-broadcast (step-0) AP."""
    return bass.AP(tensor=ap.tensor, offset=ap.offset, ap=[ap.ap[0], [0, n]])


def build_nc():
    nc = bacc.Bacc(
        "TRN2",
        target_bir_lowering=False,
        debug=False,
        enable_asserts=False,
        num_devices=NCORES,
    )

    xt_d = nc.dram_tensor("xt", [D, R], S_DT, kind="ExternalInput")
    xadd_d = nc.dram_tensor("xadd", [R, D], F32, kind="ExternalInput")
    w1_d = nc.dram_tensor("w1", [128, DK * 2 * NH * 128], S_DT, kind="ExternalInput")
    w2_d = nc.dram_tensor("w2", [128, 2 * NH * D], S_DT, kind="ExternalInput")
    cos_d = nc.dram_tensor("cost", [128, NH * L], EW_DT, kind="ExternalInput")
    sin_d = nc.dram_tensor("sint", [128, NH * L], EW_DT, kind="ExternalInput")
    rho_d = nc.dram_tensor("rho", [128, NH], F32, kind="ExternalInput")
    mask_d = nc.dram_tensor("maskc", [128, RT], F32, kind="ExternalInput")
    y_d = nc.dram_tensor("y", [R, D], F32, kind="ExternalOutput")

    with tile.TileContext(nc) as tc:
        with (
            tc.tile_pool(name="consts", bufs=1) as consts,
            tc.tile_pool(name="wplanes", bufs=4) as wplanes,
            tc.tile_pool(name="xtp", bufs=10) as xt_p,
            tc.tile_pool(name="uvp", bufs=16) as uv_p,
            tc.tile_pool(name="x2p", bufs=2) as x2_p,
            tc.tile_pool(name="sp", bufs=10) as s_p,
            tc.tile_pool(name="yp", bufs=3) as y_p,
            tc.tile_pool(name="ps_it", bufs=5, space="PSUM") as ps_it,
            tc.tile_pool(name="ps_y", bufs=3, space="PSUM") as ps_y,
        ):
            w1_sb = []
            for k in range(DK):
                w1k = consts.tile([128, 2 * NH * 128], S_DT, tag=f"w1_{k}",
                                  name=f"w1sb_{k}")
                for q in range(2):
                    nc.scalar.dma_start(
                        w1k[q * 64:(q + 1) * 64, :],
                        w1_d.ap()[q * 64:(q + 1) * 64, k * 512:(k + 1) * 512],
                    )
                w1_sb.append(w1k)
            w2_sb = consts.tile([128, 2 * NH * D], S_DT, tag="w2")
            for q in range(2):
                nc.scalar.dma_start(w2_sb[q * 64:(q + 1) * 64, :],
                                  w2_d.ap()[q * 64:(q + 1) * 64, :])
            cos_sb = consts.tile([128, NH * L], EW_DT, tag="cos")
            nc.gpsimd.dma_start(cos_sb[:], cos_d.ap())
            sin_sb = consts.tile([128, NH * L], EW_DT, tag="sin")
            nc.gpsimd.dma_start(sin_sb[:], sin_d.ap())
            rho_sb = consts.tile([128, NH], F32, tag="rho")
            nc.gpsimd.dma_start(rho_sb[:], rho_d.ap())
            mask_sb = consts.tile([128, RT], F32, tag="maskc")
            nc.gpsimd.dma_start(mask_sb[:], mask_d.ap())

            # full-width scan outputs in SCHEDULE order: schedule slot i
            # covers chunk rc_order[i] at cols [i*512, (i+1)*512). The two
            # per-batch scan chains are interleaved (even slots = batch 0).
            w_pl = [
                [wplanes.tile([128, R], EW_DT, tag="wpl", name=f"w_{p}_{h}")
                 for h in range(NH)]
                for p in range(2)
            ]

            rc_order = [0, 4, 1, 5, 2, 6, 3, 7]

            def emit_phase_d(pj, s_ch, last):
                # phase-D for schedule pair pj = chunks rc_order[2pj], rc_order[2pj+1]
                for sub in range(2):
                    rc = rc_order[2 * pj + sub]
                    sublast = last and sub == 1
                    rows = slice(rc * 512, (rc + 1) * 512)
                    xadd_r = xadd_d.ap()[rows, :].rearrange(
                        "(a p) d -> p a d", p=128)
                    ysb = y_p.tile([128, 4 * D], F32, tag="ysb", name=f"ysb_{rc}")
                    if sublast:
                        x2 = x2_p.tile([128, 4 * D], F32, tag="x2", name=f"x2_{rc}")
                        nc.sync.dma_start(
                            x2[:].rearrange("p (a d) -> p a d", a=4), xadd_r)
                    for rt2 in range(4):
                        rt = rc * 4 + rt2
                        scol = sub * 512 + rt2 * 128
                        ps = ps_y.tile([128, D], F32, tag="y", name=f"ys_{rt}")
                        first = True
                        for plane in range(2):
                            for half in range(NH):
                                nc.tensor.matmul(
                                    ps[:],
                                    s_ch[plane][half][:, scol:scol + 128],
                                    w2_sb[:, (plane * 2 + half) * D:(plane * 2 + half + 1) * D],
                                    start=first,
                                    stop=(plane == 1 and half == NH - 1),
                                )
                                first = False
                        ycol = slice(rt2 * D, (rt2 + 1) * D)
                        if sublast:
                            nc.vector.scalar_tensor_tensor(
                                out=ysb[:, ycol], in0=ps[:],
                                scalar=mask_sb[:, rt:rt + 1],
                                in1=x2[:, ycol],
                                op0=AluOp.mult, op1=AluOp.add,
                            )
                        else:
                            nc.scalar.activation(
                                ysb[:, ycol], ps[:], ACT_COPY,
                                scale=mask_sb[:, rt:rt + 1],
                            )
                    if not sublast:
                        nc.gpsimd.dma_start(
                            ysb[:].rearrange("p (a d) -> p a d", a=4),
                            xadd_r, accum_op=AluOp.add)
                    nc.sync.dma_start(
                        y_d.ap()[rows, :].rearrange("(a p) d -> p a d", p=128),
                        ysb[:].rearrange("p (a d) -> p a d", a=4),
                    )

            def bcast2(ap512):
                # [128, 512] table slice -> [128, 2, 512] repeat-broadcast
                return bass.AP(tensor=ap512.tensor, offset=ap512.offset,
                               ap=[ap512.ap[0], [0, 2], ap512.ap[1]])

            pending = None
            for pj in range(4):
                ccol = pj * 512          # l-offset within batch (same for pair)
                it_pair = [[None] * NH for _ in range(2)]
                for sub in range(2):
                    rc = rc_order[2 * pj + sub]
                    dcol = slice(rc * 512, rc * 512 + 512)
                    xt = []
                    for k in range(DK):
                        t = xt_p.tile([128, 512], S_DT, tag="xt",
                                      name=f"xt_{rc}_{k}")
                        nc.sync.dma_start(
                            t[:], xt_d.ap()[k * 128:(k + 1) * 128, dcol])
                        xt.append(t)
                    for plane in range(2):
                        for half in range(NH):
                            ps = ps_it.tile([128, 512], F32, tag="it",
                                            name=f"it_{rc}_{plane}_{half}")
                            for k in range(DK):
                                col = (plane * 2 + half) * 128
                                nc.tensor.matmul(
                                    ps[:],
                                    w1_sb[k][:, col:col + 128],
                                    xt[k][:],
                                    start=(k == 0),
                                    stop=(k == DK - 1),
                                )
                            it_pair[plane][half] = (it_pair[plane][half] or []) \
                                if isinstance(it_pair[plane][half], list) else []
                            it_pair[plane][half].append(ps)

                # software-pipelined phase-D of the previous pair sits here so
                # the PE has dense work while the DVE processes this pair
                if pending is not None:
                    emit_phase_d(*pending, last=False)

                s_ch = [[None] * NH for _ in range(2)]
                for half in range(NH):
                    cs512 = cos_sb[:, half * L + ccol:half * L + ccol + 512]
                    sn512 = sin_sb[:, half * L + ccol:half * L + ccol + 512]
                    cs = bcast2(cs512)
                    sn = bcast2(sn512)
                    ure = uv_p.tile([128, 1024], EW_DT, tag="uv",
                                    name=f"ure_{pj}_{half}")
                    uim = uv_p.tile([128, 1024], EW_DT, tag="uv",
                                    name=f"uim_{pj}_{half}")

                    def pview(t):
                        return t[:].rearrange("p (a d) -> p a d", a=2)

                    vre = uv_p.tile([128, 1024], EW_DT, tag="uv", name=f"vre_{pj}_{half}")
                    vim = uv_p.tile([128, 1024], EW_DT, tag="uv", name=f"vim_{pj}_{half}")
                    if pj == 0:
                        # first pair: per-chunk rotation so the DVE starts as
                        # soon as the FIRST chunk's matmuls land
                        for sub in range(2):
                            sl = slice(sub * 512, (sub + 1) * 512)
                            nc.scalar.activation(
                                ure[:, sl], it_pair[0][half][sub][:], ACT_COPY)
                            nc.scalar.activation(
                                uim[:, sl], it_pair[1][half][sub][:], ACT_COPY)
                            t1 = uv_p.tile([128, 512], EW_DT, tag="uvs",
                                           name=f"t1_{pj}_{half}_{sub}")
                            nc.vector.tensor_tensor(t1[:], ure[:, sl], cs512,
                                                    op=AluOp.mult)
                            t2 = uv_p.tile([128, 512], EW_DT, tag="uvs",
                                           name=f"t2_{pj}_{half}_{sub}")
                            nc.vector.tensor_tensor(t2[:], uim[:, sl], sn512,
                                                    op=AluOp.mult)
                            t3 = uv_p.tile([128, 512], EW_DT, tag="uvs",
                                           name=f"t3_{pj}_{half}_{sub}")
                            nc.vector.tensor_tensor(t3[:], uim[:, sl], cs512,
                                                    op=AluOp.mult)
                            t4 = uv_p.tile([128, 512], EW_DT, tag="uvs",
                                           name=f"t4_{pj}_{half}_{sub}")
                            nc.vector.tensor_tensor(t4[:], ure[:, sl], sn512,
                                                    op=AluOp.mult)
                            nc.vector.tensor_add(vre[:, sl], t1[:], t2[:])
                            nc.vector.tensor_sub(vim[:, sl], t3[:], t4[:])
                    else:
                        for sub in range(2):
                            sl = slice(sub * 512, (sub + 1) * 512)
                            nc.scalar.activation(
                                ure[:, sl], it_pair[0][half][sub][:], ACT_COPY)
                            nc.scalar.activation(
                                uim[:, sl], it_pair[1][half][sub][:], ACT_COPY)
                        # v = e^{-i theta l} * u, both chunks at once
                        t1 = uv_p.tile([128, 1024], EW_DT, tag="uv", name=f"t1_{pj}_{half}")
                        nc.vector.tensor_tensor(pview(t1), pview(ure), cs, op=AluOp.mult)
                        t2 = uv_p.tile([128, 1024], EW_DT, tag="uv", name=f"t2_{pj}_{half}")
                        nc.vector.tensor_tensor(pview(t2), pview(uim), sn, op=AluOp.mult)
                        t3 = uv_p.tile([128, 1024], EW_DT, tag="uv", name=f"t3_{pj}_{half}")
                        nc.vector.tensor_tensor(pview(t3), pview(uim), cs, op=AluOp.mult)
                        t4 = uv_p.tile([128, 1024], EW_DT, tag="uv", name=f"t4_{pj}_{half}")
                        nc.vector.tensor_tensor(pview(t4), pview(ure), sn, op=AluOp.mult)
                        nc.vector.tensor_add(vre[:], t1[:], t2[:])
                        nc.vector.tensor_sub(vim[:], t3[:], t4[:])

                    # per-chunk chained scans (independent chains per sub)
                    rho_b = _bcast_cols(rho_sb[:, half:half + 1], 512)
                    for plane, vch in ((0, vre), (1, vim)):
                        wp = w_pl[plane][half]
                        for sub in range(2):
                            si = 2 * pj + sub
                            scol = slice(si * 512, (si + 1) * 512)
                            if pj == 0:
                                init = 0.0
                            else:
                                prev = (si - 2) * 512 + 511
                                init = wp[:, prev:prev + 1]
                            nc.vector.tensor_tensor_scan(
                                out=wp[:, scol],
                                data0=rho_b,
                                data1=vch[:, sub * 512:(sub + 1) * 512],
                                initial=init,
                                op0=AluOp.mult,
                                op1=AluOp.add,
                            )

                    # s = e^{+i theta l} * w, both chunks at once (adjacent
                    # cols thanks to schedule-ordered w planes)
                    pcol = slice(2 * pj * 512, (2 * pj + 2) * 512)
                    wre = w_pl[0][half][:, pcol].rearrange("p (a d) -> p a d", a=2)
                    wim = w_pl[1][half][:, pcol].rearrange("p (a d) -> p a d", a=2)
                    q1 = uv_p.tile([128, 1024], EW_DT, tag="uv", name=f"q1_{pj}_{half}")
                    nc.vector.tensor_tensor(pview(q1), wre, cs, op=AluOp.mult)
                    q2 = uv_p.tile([128, 1024], EW_DT, tag="uv", name=f"q2_{pj}_{half}")
                    nc.vector.tensor_tensor(pview(q2), wim, sn, op=AluOp.mult)
                    q3 = uv_p.tile([128, 1024], EW_DT, tag="uv", name=f"q3_{pj}_{half}")
                    nc.vector.tensor_tensor(pview(q3), wim, cs, op=AluOp.mult)
                    q4 = uv_p.tile([128, 1024], EW_DT, tag="uv", name=f"q4_{pj}_{half}")
                    nc.vector.tensor_tensor(pview(q4), wre, sn, op=AluOp.mult)
                    sre = s_p.tile([128, 1024], S_DT, tag="sch",
                                   name=f"sre_{pj}_{half}")
                    nc.vector.tensor_sub(sre[:], q1[:], q2[:])
                    sim = s_p.tile([128, 1024], S_DT, tag="sch",
                                   name=f"sim_{pj}_{half}")
                    nc.vector.tensor_add(sim[:], q3[:], q4[:])
                    s_ch[0][half] = sre
                    s_ch[1][half] = sim

                pending = (pj, s_ch)

            emit_phase_d(*pending, last=True)

    nc.compile()
    return nc


_NC_CACHE = {}


def _get_nc():
    if "nc" not in _NC_CACHE:
        _NC_CACHE["nc"] = build_nc()
    return _NC_CACHE["nc"]


def _host_prep(lengths, lambda_real_log, lambda_imag, log_dt, B_re, B_im, C_re, C_im):
    lam_re = -np.exp(np.asarray(lambda_real_log, np.float64))
    lam_im = np.asarray(lambda_imag, np.float64)
    dtv = np.log1p(np.exp(np.float64(log_dt))) + 1e-4
    rho = np.exp(dtv * lam_re)                       # [N]
    theta = dtv * lam_im                             # [N]
    lam = lam_re + 1j * lam_im
    abar = np.exp(dtv * lam)
    bb = ((abar - 1.0) / lam)[:, None] * (
        np.asarray(B_re, np.float64) + 1j * np.asarray(B_im, np.float64)
    )                                                # [N, D] complex
    bb_planes = (np.ascontiguousarray(bb.real), np.ascontiguousarray(bb.imag))

    import ml_dtypes as _mld2
    w1 = np.empty((128, DK * 2 * NH * 128), np.dtype(_mld2.bfloat16))
    for k in range(DK):
        for plane in range(2):
            for half in range(NH):
                col = ((k * 2 + plane) * 2 + half) * 128
                w1[:, col:col + 128] = bb_planes[plane][
                    half * 128:(half + 1) * 128, k * 128:(k + 1) * 128
                ].T.astype(np.float32)

    import ml_dtypes as _mld
    w2 = np.empty((128, 2 * NH * D), np.dtype(_mld.bfloat16))
    c_planes = (np.asarray(C_re, np.float64), -np.asarray(C_im, np.float64))  # [D, N]
    for plane in range(2):
        for half in range(NH):
            col = (plane * 2 + half) * D
            w2[:, col:col + D] = c_planes[plane][
                :, half * 128:(half + 1) * 128
            ].T.astype(np.float32)

    import ml_dtypes
    bf16 = np.dtype(ml_dtypes.bfloat16)
    l_idx = np.arange(L, dtype=np.float64)
    cosst = np.empty((128, NH * L), bf16)
    sinst = np.empty((128, NH * L), bf16)
    for half in range(NH):
        ph = theta[half * 128:(half + 1) * 128, None] * l_idx[None, :]
        cosst[:, half * L:(half + 1) * L] = np.cos(ph).astype(bf16)
        sinst[:, half * L:(half + 1) * L] = np.sin(ph).astype(bf16)

    rho_in = np.empty((128, NH), np.float32)
    for half in range(NH):
        rho_in[:, half] = rho[half * 128:(half + 1) * 128]

    mask_bl = (np.arange(L)[None, :] < np.asarray(lengths)[:, None]).astype(np.float32)  # [B, L]
    return w1, w2, cosst, sinst, rho_in, mask_bl


def _make_in_maps(x, xadd, w1, w2, cosst, sinst, rho_in, mask_bl):
    in_maps = []
    for c in range(NCORES):
        bsl = slice(c * BL, (c + 1) * BL)
        maskc = np.ascontiguousarray(mask_bl[bsl].reshape(R).reshape(RT, 128).T)
        import ml_dtypes as _mld3
        xt = np.ascontiguousarray(x[bsl].reshape(R, D).T.astype(np.dtype(_mld3.bfloat16)))
        in_maps.append({
            "xt": xt,
            "xadd": np.ascontiguousarray(xadd[bsl].reshape(R, D)),
            "w1": w1, "w2": w2, "cost": cosst, "sint": sinst,
            "rho": rho_in, "maskc": maskc,
        })
    return in_maps


def kernel(x, lengths, lambda_real_log, lambda_imag, log_dt, B_re, B_im, C_re, C_im,
           D_weight):
    x = np.asarray(x, np.float32)
    w1, w2, cosst, sinst, rho_in, mask_bl = _host_prep(
        lengths, lambda_real_log, lambda_imag, log_dt, B_re, B_im, C_re, C_im
    )

    Dw = np.asarray(D_weight, np.float32)
    if Dw.shape == (D, D) and np.array_equal(Dw, np.eye(D, dtype=np.float32)):
        xd = x
    else:
        xd = (x.reshape(B * L, D) @ Dw.T.astype(np.float32)).reshape(B, L, D)
    xadd = xd * mask_bl[:, :, None]  # [B, L, D]

    nc = _get_nc()
    in_maps = _make_in_maps(x, xadd, w1, w2, cosst, sinst, rho_in, mask_bl)

    last_err = None
    for attempt in range(4):  # device errors are occasionally transient under axon
        try:
            res = run_bass_kernel_spmd(nc, in_maps, core_ids=list(range(NCORES)))
            break
        except Exception as e:  # noqa: BLE001
            last_err = e
            time.sleep(5 * (attempt + 1))
    else:
        raise last_err
    y = np.empty((B, L, D), np.float32)
    for c in range(NCORES):
        y[c * BL:(c + 1) * BL] = res.results[c]["y"].reshape(BL, L, D)
    return y


# revision 37
# speedup vs baseline: 1.0649x; 1.0649x over previous
"""Trainium2 Bass kernel for DiagonalS5SSM.

Math (per batch b, with the reference's where(valid,...) elided — valid is a
prefix mask in l and the output is masked by the same prefix, so the frozen
tail states never reach the output):

    it[l, n]  = sum_d x[b, l, d] * bbar[n, d]          (complex)
    s[l, n]   = abar[n] * s[l-1, n] + it[l, n]         (complex scan)
    y[b, l, :] = mask[l] * (Re(s[l] @ c^T) + x[b, l] @ D^T)

The complex scan is decoupled into two real scans via polar form
abar = rho * e^{i theta} (rho < 1, so no dynamic-range blowup):

    v[l] = e^{-i theta l} * it[l]       (elementwise rotation)
    w[l] = rho * w[l-1] + v[l]          (HW tensor_tensor_scan per plane)
    s[l] = e^{+i theta l} * w[l]        (rotation back)

Sharding: data-parallel over batch, 2 batches per core, no collectives.
Matmuls and elementwise rotations run in bf16 (separate LDWEIGHTS pipelines
under the matmul via the PE reorder window; DVE gets its 2x packed mode);
the scan coefficient rho stays fp32 (decay errors would compound over 2048
steps) and the scan state is fp32 internally in hardware. x is
pre-transposed on the host so the contraction dim lands on partitions
without PE transposes. The whole pipeline is chunk-streamed in 512-row
chunks with the two per-batch scan chains interleaved; the scan chains
across chunks via initial=prev[:, -1:]. The final y = mask*resp + x@D^T
uses an ACT masked copy plus a gpsimd accumulate-DMA of the host-prepared
x@D^T term (identity D short-circuits the host matmul).
"""

import time

import numpy as np

import concourse.bass as bass
import concourse.tile as tile
from concourse import bacc, mybir
from concourse.bass_utils import run_bass_kernel_spmd

B, L, D, N = 16, 2048, 512, 256
NCORES = 8
BL = B // NCORES          # batches per core
R = BL * L                # rows per core (r = b*L + l)
NH = N // 128             # n-halves
DK = D // 128             # d-chunks
RT = R // 128             # 128-row tiles per core
RCH = R // 512            # 512-row chunks per core
CPB = L // 512            # chunks per batch

F32 = mybir.dt.float32
BF16 = mybir.dt.bfloat16
MM_DT = mybir.dt.float32r  # matmul operand dtype (float32r: 1 cyc/row)
EW_DT = BF16               # elementwise/rotation dtype (2x DVE mode)
S_DT = BF16                # s-plane / stage-5 matmul dtype

AluOp = mybir.AluOpType
ACT_COPY = mybir.ActivationFunctionType.Copy


def _bcast_cols(ap: bass.AP, n: int) -> bass.AP:
    """[128, 1] AP -> [128, n] free-broadcast (step-0) AP."""
    return bass.AP(tensor=ap.tensor, offset=ap.offset, ap=[ap.ap[0], [0, n]])


def build_nc():
    nc = bacc.Bacc(
        "TRN2",
        target_bir_lowering=False,
        debug=False,
        enable_asserts=False,
        num_devices=NCORES,
    )

    xt_d = nc.dram_tensor("xt", [D, R], S_DT, kind="ExternalInput")
    xadd_d = nc.dram_tensor("xadd", [R, D], F32, kind="ExternalInput")
    w1_d = nc.dram_tensor("w1", [128, DK * 2 * NH * 128], S_DT, kind="ExternalInput")
    w2_d = nc.dram_tensor("w2", [128, 2 * NH * D], S_DT, kind="ExternalInput")
    cos_d = nc.dram_tensor("cost", [128, NH * L], EW_DT, kind="ExternalInput")
    sin_d = nc.dram_tensor("sint", [128, NH * L], EW_DT, kind="ExternalInput")
    rho_d = nc.dram_tensor("rho", [128, NH], F32, kind="ExternalInput")
    mask_d = nc.dram_tensor("maskc", [128, RT], F32, kind="ExternalInput")
    y_d = nc.dram_tensor("y", [R, D], F32, kind="ExternalOutput")

    with tile.TileContext(nc) as tc:
        with (
            tc.tile_pool(name="consts", bufs=1) as consts,
            tc.tile_pool(name="wplanes", bufs=4) as wplanes,
            tc.tile_pool(name="xtp", bufs=10) as xt_p,
            tc.tile_pool(name="uvp", bufs=16) as uv_p,
            tc.tile_pool(name="x2p", bufs=2) as x2_p,
            tc.tile_pool(name="sp", bufs=10) as s_p,
            tc.tile_pool(name="yp", bufs=3) as y_p,
            tc.tile_pool(name="ps_it", bufs=5, space="PSUM") as ps_it,
            tc.tile_pool(name="ps_y", bufs=3, space="PSUM") as ps_y,
        ):
            w1_sb = []
            for k in range(DK):
                w1k = consts.tile([128, 2 * NH * 128], S_DT, tag=f"w1_{k}",
                                  name=f"w1sb_{k}")
                for q in range(2):
                    nc.scalar.dma_start(
                        w1k[q * 64:(q + 1) * 64, :],
                        w1_d.ap()[q * 64:(q + 1) * 64, k * 512:(k + 1) * 512],
                    )
                w1_sb.append(w1k)
            w2_sb = consts.tile([128, 2 * NH * D], S_DT, tag="w2")
            for q in range(2):
                nc.scalar.dma_start(w2_sb[q * 64:(q + 1) * 64, :],
                                  w2_d.ap()[q * 64:(q + 1) * 64, :])
            cos_sb = consts.tile([128, NH * L], EW_DT, tag="cos")
            nc.gpsimd.dma_start(cos_sb[:], cos_d.ap())
            sin_sb = consts.tile([128, NH * L], EW_DT, tag="sin")
            nc.gpsimd.dma_start(sin_sb[:], sin_d.ap())
            rho_sb = consts.tile([128, NH], F32, tag="rho")
            nc.gpsimd.dma_start(rho_sb[:], rho_d.ap())
            mask_sb = consts.tile([128, RT], F32, tag="maskc")
            nc.gpsimd.dma_start(mask_sb[:], mask_d.ap())

            # full-width scan outputs in SCHEDULE order: schedule slot i
            # covers chunk rc_order[i] at cols [i*512, (i+1)*512). The two
            # per-batch scan chains are interleaved (even slots = batch 0).
            w_pl = [
                [wplanes.tile([128, R], EW_DT, tag="wpl", name=f"w_{p}_{h}")
                 for h in range(NH)]
                for p in range(2)
            ]

            rc_order = [0, 4, 1, 5, 2, 6, 3, 7]

            def emit_phase_d(pj, s_ch, last):
                # phase-D for schedule pair pj = chunks rc_order[2pj], rc_order[2pj+1]
                for sub in range(2):
                    rc = rc_order[2 * pj + sub]
                    sublast = last
                    rows = slice(rc * 512, (rc + 1) * 512)
                    xadd_r = xadd_d.ap()[rows, :].rearrange(
                        "(a p) d -> p a d", p=128)
                    ysb = y_p.tile([128, 4 * D], F32, tag="ysb", name=f"ysb_{rc}")
                    if sublast:
                        x2 = x2_p.tile([128, 4 * D], F32, tag="x2", name=f"x2_{rc}")
                        nc.sync.dma_start(
                            x2[:].rearrange("p (a d) -> p a d", a=4), xadd_r)
                    for rt2 in range(4):
                        rt = rc * 4 + rt2
                        scol = sub * 512 + rt2 * 128
                        ps = ps_y.tile([128, D], F32, tag="y", name=f"ys_{rt}")
                        first = True
                        for plane in range(2):
                            for half in range(NH):
                                nc.tensor.matmul(
                                    ps[:],
                                    s_ch[plane][half][:, scol:scol + 128],
                                    w2_sb[:, (plane * 2 + half) * D:(plane * 2 + half + 1) * D],
                                    start=first,
                                    stop=(plane == 1 and half == NH - 1),
                                )
                                first = False
                        ycol = slice(rt2 * D, (rt2 + 1) * D)
                        if sublast:
                            nc.vector.scalar_tensor_tensor(
                                out=ysb[:, ycol], in0=ps[:],
                                scalar=mask_sb[:, rt:rt + 1],
                                in1=x2[:, ycol],
                                op0=AluOp.mult, op1=AluOp.add,
                            )
                        else:
                            nc.scalar.activation(
                                ysb[:, ycol], ps[:], ACT_COPY,
                                scale=mask_sb[:, rt:rt + 1],
                            )
                    if not sublast:
                        nc.gpsimd.dma_start(
                            ysb[:].rearrange("p (a d) -> p a d", a=4),
                            xadd_r, accum_op=AluOp.add)
                    nc.sync.dma_start(
                        y_d.ap()[rows, :].rearrange("(a p) d -> p a d", p=128),
                        ysb[:].rearrange("p (a d) -> p a d", a=4),
                    )

            def bcast2(ap512):
                # [128, 512] table slice -> [128, 2, 512] repeat-broadcast
                return bass.AP(tensor=ap512.tensor, offset=ap512.offset,
                               ap=[ap512.ap[0], [0, 2], ap512.ap[1]])

            pending = None
            for pj in range(4):
                ccol = pj * 512          # l-offset within batch (same for pair)
                it_pair = [[None] * NH for _ in range(2)]
                for sub in range(2):
                    rc = rc_order[2 * pj + sub]
                    dcol = slice(rc * 512, rc * 512 + 512)
                    xt = []
                    for k in range(DK):
                        t = xt_p.tile([128, 512], S_DT, tag="xt",
                                      name=f"xt_{rc}_{k}")
                        nc.sync.dma_start(
                            t[:], xt_d.ap()[k * 128:(k + 1) * 128, dcol])
                        xt.append(t)
                    for plane in range(2):
                        for half in range(NH):
                            ps = ps_it.tile([128, 512], F32, tag="it",
                                            name=f"it_{rc}_{plane}_{half}")
                            for k in range(DK):
                                col = (plane * 2 + half) * 128
                                nc.tensor.matmul(
                                    ps[:],
                                    w1_sb[k][:, col:col + 128],
                                    xt[k][:],
                                    start=(k == 0),
                                    stop=(k == DK - 1),
                                )
                            it_pair[plane][half] = (it_pair[plane][half] or []) \
                                if isinstance(it_pair[plane][half], list) else []
                            it_pair[plane][half].append(ps)

                # software-pipelined phase-D of the previous pair sits here so
                # the PE has dense work while the DVE processes this pair
                if pending is not None:
                    emit_phase_d(*pending, last=False)

                s_ch = [[None] * NH for _ in range(2)]
                for half in range(NH):
                    cs512 = cos_sb[:, half * L + ccol:half * L + ccol + 512]
                    sn512 = sin_sb[:, half * L + ccol:half * L + ccol + 512]
                    cs = bcast2(cs512)
                    sn = bcast2(sn512)
                    ure = uv_p.tile([128, 1024], EW_DT, tag="uv",
                                    name=f"ure_{pj}_{half}")
                    uim = uv_p.tile([128, 1024], EW_DT, tag="uv",
                                    name=f"uim_{pj}_{half}")

                    def pview(t):
                        return t[:].rearrange("p (a d) -> p a d", a=2)

                    vre = uv_p.tile([128, 1024], EW_DT, tag="uv", name=f"vre_{pj}_{half}")
                    vim = uv_p.tile([128, 1024], EW_DT, tag="uv", name=f"vim_{pj}_{half}")
                    if pj == 0:
                        # first pair: per-chunk rotation so the DVE starts as
                        # soon as the FIRST chunk's matmuls land
                        for sub in range(2):
                            sl = slice(sub * 512, (sub + 1) * 512)
                            nc.scalar.activation(
                                ure[:, sl], it_pair[0][half][sub][:], ACT_COPY)
                            nc.scalar.activation(
                                uim[:, sl], it_pair[1][half][sub][:], ACT_COPY)
                            t1 = uv_p.tile([128, 512], EW_DT, tag="uvs",
                                           name=f"t1_{pj}_{half}_{sub}")
                            nc.vector.tensor_tensor(t1[:], ure[:, sl], cs512,
                                                    op=AluOp.mult)
                            t2 = uv_p.tile([128, 512], EW_DT, tag="uvs",
                                           name=f"t2_{pj}_{half}_{sub}")
                            nc.vector.tensor_tensor(t2[:], uim[:, sl], sn512,
                                                    op=AluOp.mult)
                            t3 = uv_p.tile([128, 512], EW_DT, tag="uvs",
                                           name=f"t3_{pj}_{half}_{sub}")
                            nc.vector.tensor_tensor(t3[:], uim[:, sl], cs512,
                                                    op=AluOp.mult)
                            t4 = uv_p.tile([128, 512], EW_DT, tag="uvs",
                                           name=f"t4_{pj}_{half}_{sub}")
                            nc.vector.tensor_tensor(t4[:], ure[:, sl], sn512,
                                                    op=AluOp.mult)
                            nc.vector.tensor_add(vre[:, sl], t1[:], t2[:])
                            nc.vector.tensor_sub(vim[:, sl], t3[:], t4[:])
                    else:
                        for sub in range(2):
                            sl = slice(sub * 512, (sub + 1) * 512)
                            nc.scalar.activation(
                                ure[:, sl], it_pair[0][half][sub][:], ACT_COPY)
                            nc.scalar.activation(
                                uim[:, sl], it_pair[1][half][sub][:], ACT_COPY)
                        # v = e^{-i theta l} * u, both chunks at once
                        t1 = uv_p.tile([128, 1024], EW_DT, tag="uv", name=f"t1_{pj}_{half}")
                        nc.vector.tensor_tensor(pview(t1), pview(ure), cs, op=AluOp.mult)
                        t2 = uv_p.tile([128, 1024], EW_DT, tag="uv", name=f"t2_{pj}_{half}")
                        nc.vector.tensor_tensor(pview(t2), pview(uim), sn, op=AluOp.mult)
                        t3 = uv_p.tile([128, 1024], EW_DT, tag="uv", name=f"t3_{pj}_{half}")
                        nc.vector.tensor_tensor(pview(t3), pview(uim), cs, op=AluOp.mult)
                        t4 = uv_p.tile([128, 1024], EW_DT, tag="uv", name=f"t4_{pj}_{half}")
                        nc.vector.tensor_tensor(pview(t4), pview(ure), sn, op=AluOp.mult)
                        nc.vector.tensor_add(vre[:], t1[:], t2[:])
                        nc.vector.tensor_sub(vim[:], t3[:], t4[:])

                    # per-chunk chained scans (independent chains per sub)
                    rho_b = _bcast_cols(rho_sb[:, half:half + 1], 512)
                    for plane, vch in ((0, vre), (1, vim)):
                        wp = w_pl[plane][half]
                        for sub in range(2):
                            si = 2 * pj + sub
                            scol = slice(si * 512, (si + 1) * 512)
                            if pj == 0:
                                init = 0.0
                            else:
                                prev = (si - 2) * 512 + 511
                                init = wp[:, prev:prev + 1]
                            nc.vector.tensor_tensor_scan(
                                out=wp[:, scol],
                                data0=rho_b,
                                data1=vch[:, sub * 512:(sub + 1) * 512],
                                initial=init,
                                op0=AluOp.mult,
                                op1=AluOp.add,
                            )

                    # s = e^{+i theta l} * w, both chunks at once (adjacent
                    # cols thanks to schedule-ordered w planes)
                    pcol = slice(2 * pj * 512, (2 * pj + 2) * 512)
                    wre = w_pl[0][half][:, pcol].rearrange("p (a d) -> p a d", a=2)
                    wim = w_pl[1][half][:, pcol].rearrange("p (a d) -> p a d", a=2)
                    q1 = uv_p.tile([128, 1024], EW_DT, tag="uv", name=f"q1_{pj}_{half}")
                    nc.vector.tensor_tensor(pview(q1), wre, cs, op=AluOp.mult)
                    q2 = uv_p.tile([128, 1024], EW_DT, tag="uv", name=f"q2_{pj}_{half}")
                    nc.vector.tensor_tensor(pview(q2), wim, sn, op=AluOp.mult)
                    q3 = uv_p.tile([128, 1024], EW_DT, tag="uv", name=f"q3_{pj}_{half}")
                    nc.vector.tensor_tensor(pview(q3), wim, cs, op=AluOp.mult)
                    q4 = uv_p.tile([128, 1024], EW_DT, tag="uv", name=f"q4_{pj}_{half}")
                    nc.vector.tensor_tensor(pview(q4), wre, sn, op=AluOp.mult)
                    sre = s_p.tile([128, 1024], S_DT, tag="sch",
                                   name=f"sre_{pj}_{half}")
                    nc.vector.tensor_sub(sre[:], q1[:], q2[:])
                    sim = s_p.tile([128, 1024], S_DT, tag="sch",
                                   name=f"sim_{pj}_{half}")
                    nc.vector.tensor_add(sim[:], q3[:], q4[:])
                    s_ch[0][half] = sre
                    s_ch[1][half] = sim

                pending = (pj, s_ch)

            emit_phase_d(*pending, last=True)

    nc.compile()
    return nc


_NC_CACHE = {}


def _get_nc():
    if "nc" not in _NC_CACHE:
        _NC_CACHE["nc"] = build_nc()
    return _NC_CACHE["nc"]


def _host_prep(lengths, lambda_real_log, lambda_imag, log_dt, B_re, B_im, C_re, C_im):
    lam_re = -np.exp(np.asarray(lambda_real_log, np.float64))
    lam_im = np.asarray(lambda_imag, np.float64)
    dtv = np.log1p(np.exp(np.float64(log_dt))) + 1e-4
    rho = np.exp(dtv * lam_re)                       # [N]
    theta = dtv * lam_im                             # [N]
    lam = lam_re + 1j * lam_im
    abar = np.exp(dtv * lam)
    bb = ((abar - 1.0) / lam)[:, None] * (
        np.asarray(B_re, np.float64) + 1j * np.asarray(B_im, np.float64)
    )                                                # [N, D] complex
    bb_planes = (np.ascontiguousarray(bb.real), np.ascontiguousarray(bb.imag))

    import ml_dtypes as _mld2
    w1 = np.empty((128, DK * 2 * NH * 128), np.dtype(_mld2.bfloat16))
    for k in range(DK):
        for plane in range(2):
            for half in range(NH):
                col = ((k * 2 + plane) * 2 + half) * 128
                w1[:, col:col + 128] = bb_planes[plane][
                    half * 128:(half + 1) * 128, k * 128:(k + 1) * 128
                ].T.astype(np.float32)

    import ml_dtypes as _mld
    w2 = np.empty((128, 2 * NH * D), np.dtype(_mld.bfloat16))
    c_planes = (np.asarray(C_re, np.float64), -np.asarray(C_im, np.float64))  # [D, N]
    for plane in range(2):
        for half in range(NH):
            col = (plane * 2 + half) * D
            w2[:, col:col + D] = c_planes[plane][
                :, half * 128:(half + 1) * 128
            ].T.astype(np.float32)

    import ml_dtypes
    bf16 = np.dtype(ml_dtypes.bfloat16)
    l_idx = np.arange(L, dtype=np.float64)
    cosst = np.empty((128, NH * L), bf16)
    sinst = np.empty((128, NH * L), bf16)
    for half in range(NH):
        ph = theta[half * 128:(half + 1) * 128, None] * l_idx[None, :]
        cosst[:, half * L:(half + 1) * L] = np.cos(ph).astype(bf16)
        sinst[:, half * L:(half + 1) * L] = np.sin(ph).astype(bf16)

    rho_in = np.empty((128, NH), np.float32)
    for half in range(NH):
        rho_in[:, half] = rho[half * 128:(half + 1) * 128]

    mask_bl = (np.arange(L)[None, :] < np.asarray(lengths)[:, None]).astype(np.float32)  # [B, L]
    return w1, w2, cosst, sinst, rho_in, mask_bl


def _make_in_maps(x, xadd, w1, w2, cosst, sinst, rho_in, mask_bl):
    in_maps = []
    for c in range(NCORES):
        bsl = slice(c * BL, (c + 1) * BL)
        maskc = np.ascontiguousarray(mask_bl[bsl].reshape(R).reshape(RT, 128).T)
        import ml_dtypes as _mld3
        xt = np.ascontiguousarray(x[bsl].reshape(R, D).T.astype(np.dtype(_mld3.bfloat16)))
        in_maps.append({
            "xt": xt,
            "xadd": np.ascontiguousarray(xadd[bsl].reshape(R, D)),
            "w1": w1, "w2": w2, "cost": cosst, "sint": sinst,
            "rho": rho_in, "maskc": maskc,
        })
    return in_maps


def kernel(x, lengths, lambda_real_log, lambda_imag, log_dt, B_re, B_im, C_re, C_im,
           D_weight):
    x = np.asarray(x, np.float32)
    w1, w2, cosst, sinst, rho_in, mask_bl = _host_prep(
        lengths, lambda_real_log, lambda_imag, log_dt, B_re, B_im, C_re, C_im
    )

    Dw = np.asarray(D_weight, np.float32)
    if Dw.shape == (D, D) and np.array_equal(Dw, np.eye(D, dtype=np.float32)):
        xd = x
    else:
        xd = (x.reshape(B * L, D) @ Dw.T.astype(np.float32)).reshape(B, L, D)
    xadd = xd * mask_bl[:, :, None]  # [B, L, D]

    nc = _get_nc()
    in_maps = _make_in_maps(x, xadd, w1, w2, cosst, sinst, rho_in, mask_bl)

    last_err = None
    for attempt in range(4):  # device errors are occasionally transient under axon
        try:
            res = run_bass_kernel_spmd(nc, in_maps, core_ids=list(range(NCORES)))
            break
        except Exception as e:  # noqa: BLE001
            last_err = e
            time.sleep(5 * (attempt + 1))
    else:
        raise last_err
    y = np.empty((B, L, D), np.float32)
    for c in range(NCORES):
        y[c * BL:(c + 1) * BL] = res.results[c]["y"].reshape(BL, L, D)
    return y
